# revision 9
# baseline (speedup 1.0000x reference)
"""Trainium2 Bass kernel for nn_AttentionHead (B=8, S=2048, D=1024).

Sharding: data-parallel over batch — each of the 8 NeuronCores handles one
batch element; weights / biases / pos_bias are replicated.

Per-core dataflow (all matmuls run as float32r on the PE at 1 cyc/row):
  phase 1a: Q^T = (Wq/32) @ xq^T  + bq/32      -> spilled to DRAM scratch
  phase 1b: K^T = Wk @ xk^T + bk               -> SBUF resident [e, k]
  phase 1c: V   = xv @ Wv^T + bv               -> SBUF resident [k, e]
  phase 2 (per q-block of 256):
     S = Q^T.T @ K^T   (+pos_bias)  -> P = exp(S) (row-sum via accum_out)
     A = P * recip(rowsum)          -> DMA to attn_weights output
     A^T via scatter-DMA + DVE StreamTranspose -> O^T = V-tiles.T @ A^T
  phase 3: out = O^T.T @ Wo^T + bo  -> output
Transposes (x^T, A^T) use 32x32-block-scattered DMA loads + the DVE
StreamTranspose (keeps the PE free for matmuls).
"""

import numpy as np

S = 2048
D = 1024
P = 128
B = 8

_COMPILED = None


def _build_program():
    import concourse.bass as bass
    import concourse.tile as tile
    from concourse import bacc, mybir
    from contextlib import ExitStack

    f32 = mybir.dt.float32
    f32r = mybir.dt.float32r
    AF = mybir.ActivationFunctionType

    nc = bacc.Bacc("TRN2", target_bir_lowering=False, debug=False)

    # ---- per-core I/O ----
    xq = nc.dram_tensor("xq", [S, D], f32r, kind="ExternalInput").ap()
    xk = nc.dram_tensor("xk", [S, D], f32r, kind="ExternalInput").ap()
    xv = nc.dram_tensor("xv", [S, D], f32r, kind="ExternalInput").ap()
    wqt = nc.dram_tensor("wqt", [D, D], f32r, kind="ExternalInput").ap()  # Wq.T/32
    wkt = nc.dram_tensor("wkt", [D, D], f32r, kind="ExternalInput").ap()  # Wk.T
    wvt = nc.dram_tensor("wvt", [D, D], f32r, kind="ExternalInput").ap()  # Wv.T
    wot = nc.dram_tensor("wot", [D, D], f32r, kind="ExternalInput").ap()  # Wo.T
    bq8 = nc.dram_tensor("bq8", [P, 8], f32, kind="ExternalInput").ap()  # bq/32 [p, et]
    bk8 = nc.dram_tensor("bk8", [P, 8], f32, kind="ExternalInput").ap()
    bvb = nc.dram_tensor("bvb", [P, D], f32, kind="ExternalInput").ap()  # bv bcast
    bob = nc.dram_tensor("bob", [P, D], f32, kind="ExternalInput").ap()  # bo bcast
    pb = nc.dram_tensor("pb", [S, S], f32, kind="ExternalInput").ap()
    out = nc.dram_tensor("out", [S, D], f32, kind="ExternalOutput").ap()
    # attn is fp32 data; declared f32r so the A^T re-read is a valid f32r producer
    attn = nc.dram_tensor("attn", [S, S], f32r, kind="ExternalOutput").ap()

    # DRAM scratch
    qt_d = nc.dram_tensor("qt_scratch", [D, S], f32r).ap()
    ot_d = nc.dram_tensor("ot_scratch", [D, S], f32r).ap()

    ET = D // P      # 8 e-tiles
    DT = D // P      # 8 d-tiles
    NSB = S // 512   # 4 s-blocks
    NQB = S // 256   # 8 q-blocks
    KT = S // P      # 16 k-tiles

    with tile.TileContext(nc) as tc, ExitStack() as top:
        const = top.enter_context(tc.tile_pool(name="const", bufs=1))
        ktp = top.enter_context(tc.tile_pool(name="ktp", bufs=1))
        vp = top.enter_context(tc.tile_pool(name="vp", bufs=1))
        mmp = top.enter_context(tc.tile_pool(name="mmp", bufs=4, space="PSUM"))

        bq_sb = const.tile([P, 8], f32, name="bq_sb")
        nc.sync.dma_start(bq_sb[:], bq8[:])
        bk_sb = const.tile([P, 8], f32, name="bk_sb")
        nc.sync.dma_start(bk_sb[:], bk8[:])

        kt_res = ktp.tile([P, ET * S], f32r, name="kt_res")     # [e%128, et*2048+k]
        v_res = vp.tile([P, KT * D], f32r, name="v_res")        # [k%128, kt*1024+e]

        def load_xT(xlp, xsp, xtp, x_dram, sb):
            """s-block sb of x -> x^T tile [128, dt*512 + s] via scatter+DVE."""
            xT = xtp.tile([P, DT * 512], f32r, name=f"xT_{sb}", tag="xT")
            for dt in range(DT):
                L = xlp.tile([P, 512], f32, name="xL", tag="xL")
                for db in range(4):
                    src = x_dram[sb * 512:(sb + 1) * 512,
                                 dt * P + db * 32: dt * P + (db + 1) * 32]
                    nc.sync.dma_start(
                        L[db * 32:(db + 1) * 32, :],
                        src.bitcast(f32).rearrange("(sb r) c -> r sb c", r=32),
                    )
                stg = xsp.tile([P, 512], f32, name="xSt", tag="xSt")
                nc.vector.transpose(stg[:], L[:])
                nc.scalar.copy(xT[:, dt * 512:(dt + 1) * 512], stg[:])
            return xT

        # ---------------- phase 1a: Q^T -> DRAM ----------------
        with ExitStack() as ph:
            wp = ph.enter_context(tc.tile_pool(name="wp", bufs=1))
            xlp = ph.enter_context(tc.tile_pool(name="xlp", bufs=4))
            xsp = ph.enter_context(tc.tile_pool(name="xsp", bufs=2))
            xtp = ph.enter_context(tc.tile_pool(name="xtp", bufs=2))
            qks = ph.enter_context(tc.tile_pool(name="qks", bufs=1))

            w_sb = wp.tile([P, DT * D], f32r, name="wq_sb", tag="w")
            for dt in range(DT):
                nc.sync.dma_start(w_sb[:, dt * D:(dt + 1) * D], wqt[dt * P:(dt + 1) * P, :])
            for sb in range(NSB):
                xT = load_xT(xlp, xsp, xtp, xq, sb)
                for et in range(ET):
                    ps = mmp.tile([P, 512], f32, name="ps_q", tag="mm")
                    for dt in range(DT):
                        nc.tensor.matmul(
                            ps[:],
                            w_sb[:, dt * D + et * P: dt * D + (et + 1) * P],
                            xT[:, dt * 512:(dt + 1) * 512],
                            start=(dt == 0), stop=(dt == DT - 1),
                        )
                    stg = qks.tile([P, 512], f32r, name="qt_stage", tag="qs")
                    nc.scalar.activation(stg[:], ps[:], AF.Identity, bias=bq_sb[:, et:et + 1])
                    nc.sync.dma_start(qt_d[et * P:(et + 1) * P, sb * 512:(sb + 1) * 512], stg[:])

        # ---------------- phase 1b: K^T -> resident ----------------
        with ExitStack() as ph:
            wp = ph.enter_context(tc.tile_pool(name="wp_k", bufs=1))
            xlp = ph.enter_context(tc.tile_pool(name="xlp_k", bufs=4))
            xsp = ph.enter_context(tc.tile_pool(name="xsp_k", bufs=2))
            xtp = ph.enter_context(tc.tile_pool(name="xtp_k", bufs=2))

            w_sb = wp.tile([P, DT * D], f32r, name="wk_sb", tag="w")
            for dt in range(DT):
                nc.sync.dma_start(w_sb[:, dt * D:(dt + 1) * D], wkt[dt * P:(dt + 1) * P, :])
            for sb in range(NSB):
                xT = load_xT(xlp, xsp, xtp, xk, sb)
                for et in range(ET):
                    ps = mmp.tile([P, 512], f32, name="ps_k", tag="mm")
                    for dt in range(DT):
                        nc.tensor.matmul(
                            ps[:],
                            w_sb[:, dt * D + et * P: dt * D + (et + 1) * P],
                            xT[:, dt * 512:(dt + 1) * 512],
                            start=(dt == 0), stop=(dt == DT - 1),
                        )
                    nc.scalar.activation(
                        kt_res[:, et * S + sb * 512: et * S + (sb + 1) * 512],
                        ps[:], AF.Identity, bias=bk_sb[:, et:et + 1],
                    )

        # ---------------- phase 1c: V -> resident ----------------
        with ExitStack() as ph:
            wp = ph.enter_context(tc.tile_pool(name="wp_v", bufs=1))
            xlp = ph.enter_context(tc.tile_pool(name="xlp_v", bufs=4))
            xsp = ph.enter_context(tc.tile_pool(name="xsp_v", bufs=1))
            xtp = ph.enter_context(tc.tile_pool(name="xtp_v", bufs=2))
            bvp = ph.enter_context(tc.tile_pool(name="bvp", bufs=1))

            w_sb = wp.tile([P, DT * D], f32r, name="wv_sb", tag="w")
            for dt in range(DT):
                nc.sync.dma_start(w_sb[:, dt * D:(dt + 1) * D], wvt[dt * P:(dt + 1) * P, :])
            bv_sb = bvp.tile([P, D], f32, name="bv_sb")
            nc.sync.dma_start(bv_sb[:], bvb[:])
            for sb in range(NSB):
                xT = load_xT(xlp, xsp, xtp, xv, sb)
                for ss in range(4):
                    kt = sb * 4 + ss
                    for ec in range(2):
                        ps = mmp.tile([P, 512], f32, name="ps_v", tag="mm")
                        for dt in range(DT):
                            nc.tensor.matmul(
                                ps[:],
                                xT[:, dt * 512 + ss * P: dt * 512 + (ss + 1) * P],
                                w_sb[:, dt * D + ec * 512: dt * D + (ec + 1) * 512],
                                start=(dt == 0), stop=(dt == DT - 1),
                            )
                        nc.vector.tensor_add(
                            v_res[:, kt * D + ec * 512: kt * D + (ec + 1) * 512],
                            ps[:], bv_sb[:, ec * 512:(ec + 1) * 512],
                        )

        # ---------------- phase 2: attention per q-block of 256 ----------------
        with ExitStack() as ph:
            qtb = ph.enter_context(tc.tile_pool(name="qtb", bufs=2))
            pp = ph.enter_context(tc.tile_pool(name="pp", bufs=3))
            pbp = ph.enter_context(tc.tile_pool(name="pbp", bufs=3))
            plp = ph.enter_context(tc.tile_pool(name="plp", bufs=4))
            atp = ph.enter_context(tc.tile_pool(name="atp", bufs=1))
            otb = ph.enter_context(tc.tile_pool(name="otb", bufs=1))
            smallp = ph.enter_context(tc.tile_pool(name="smallp", bufs=4))
            opp = ph.enter_context(tc.tile_pool(name="opp", bufs=3, space="PSUM"))

            for qb in range(NQB):
                qtb_t = qtb.tile([P, ET * 256], f32r, name="qtb_t", tag="qtb")
                for et in range(ET):
                    nc.sync.dma_start(
                        qtb_t[:, et * 256:(et + 1) * 256],
                        qt_d[et * P:(et + 1) * P, qb * 256:(qb + 1) * 256],
                    )
                for qs in range(2):
                    q0 = qb * 256 + qs * P
                    p_t = pp.tile([P, S], f32, name=f"p_{qs}", tag="p")
                    for kc in range(4):
                        ps = mmp.tile([P, 512], f32, name="ps_s", tag="mm")
                        for et in range(ET):
                            nc.tensor.matmul(
                                ps[:],
                                qtb_t[:, et * 256 + qs * P: et * 256 + (qs + 1) * P],
                                kt_res[:, et * S + kc * 512: et * S + (kc + 1) * 512],
                                start=(et == 0), stop=(et == ET - 1),
                            )
                        pbt = pbp.tile([P, 512], f32, name="pbt", tag="pb")
                        nc.sync.dma_start(pbt[:], pb[q0:q0 + P, kc * 512:(kc + 1) * 512])
                        nc.vector.tensor_add(p_t[:, kc * 512:(kc + 1) * 512], ps[:], pbt[:])
                    rs = smallp.tile([P, 1], f32, name="rs", tag="rs")
                    nc.scalar.activation(p_t[:], p_t[:], AF.Exp, accum_out=rs[:])
                    rc = smallp.tile([P, 1], f32, name="rc", tag="rc")
                    nc.vector.reciprocal(rc[:], rs[:])
                    nc.scalar.activation(p_t[:], p_t[:], AF.Identity, scale=rc[:])
                    nc.sync.dma_start(attn[q0:q0 + P, :].bitcast(f32), p_t[:])

                # A^T tiles via scatter re-read of attn + StreamTranspose
                at_t = atp.tile([P, KT * 256], f32r, name="at_t", tag="at")
                for kt in range(KT):
                    prm = plp.tile([P, 256], f32, name="prm", tag="prm")
                    for kb in range(4):
                        src = attn[qb * 256:(qb + 1) * 256,
                                   kt * P + kb * 32: kt * P + (kb + 1) * 32]
                        nc.sync.dma_start(
                            prm[kb * 32:(kb + 1) * 32, :],
                            src.bitcast(f32).rearrange("(qb r) c -> r qb c", r=32),
                        )
                    ast = plp.tile([P, 256], f32, name="ast", tag="ast")
                    nc.vector.transpose(ast[:], prm[:])
                    nc.scalar.copy(at_t[:, kt * 256:(kt + 1) * 256], ast[:])

                otb_t = otb.tile([P, ET * 256], f32r, name="otb_t", tag="otb")
                for et in range(ET):
                    ps = opp.tile([P, 256], f32, name="ps_o", tag="op")
                    for kt in range(KT):
                        nc.tensor.matmul(
                            ps[:],
                            v_res[:, kt * D + et * P: kt * D + (et + 1) * P],
                            at_t[:, kt * 256:(kt + 1) * 256],
                            start=(kt == 0), stop=(kt == KT - 1),
                        )
                    nc.vector.tensor_copy(otb_t[:, et * 256:(et + 1) * 256], ps[:])
                for et in range(ET):
                    nc.sync.dma_start(
                        ot_d[et * P:(et + 1) * P, qb * 256:(qb + 1) * 256],
                        otb_t[:, et * 256:(et + 1) * 256],
                    )

        # ---------------- phase 3: output projection ----------------
        with ExitStack() as ph:
            wop = ph.enter_context(tc.tile_pool(name="wop", bufs=1))
            bop = ph.enter_context(tc.tile_pool(name="bop", bufs=1))
            otl = ph.enter_context(tc.tile_pool(name="otl", bufs=3))
            outs = ph.enter_context(tc.tile_pool(name="outs", bufs=4))

            wo_sb = wop.tile([P, ET * D], f32r, name="wo_sb")
            for et in range(ET):
                nc.sync.dma_start(wo_sb[:, et * D:(et + 1) * D], wot[et * P:(et + 1) * P, :])
            bo_sb = bop.tile([P, D], f32, name="bo_sb")
            nc.sync.dma_start(bo_sb[:], bob[:])
            for qp in range(NQB):  # q-pairs of 256
                ot_t = otl.tile([P, ET * 256], f32r, name="ot_t", tag="ot")
                for et in range(ET):
                    nc.gpsimd.dma_start(
                        ot_t[:, et * 256:(et + 1) * 256],
                        ot_d[et * P:(et + 1) * P, qp * 256:(qp + 1) * 256],
                    )
                for qs in range(2):
                    qt = qp * 2 + qs
                    for oc in range(2):
                        ps = mmp.tile([P, 512], f32, name="ps_f", tag="mm")
                        for et in range(ET):
                            nc.tensor.matmul(
                                ps[:],
                                ot_t[:, et * 256 + qs * P: et * 256 + (qs + 1) * P],
                                wo_sb[:, et * D + oc * 512: et * D + (oc + 1) * 512],
                                start=(et == 0), stop=(et == ET - 1),
                            )
                        os_t = outs.tile([P, 512], f32, name="os_t", tag="os")
                        nc.vector.tensor_add(os_t[:], ps[:], bo_sb[:, oc * 512:(oc + 1) * 512])
                        nc.sync.dma_start(out[qt * P:(qt + 1) * P, oc * 512:(oc + 1) * 512], os_t[:])

    nc.compile()
    return nc


def _get_compiled():
    global _COMPILED
    if _COMPILED is None:
        _COMPILED = _build_program()
    return _COMPILED


def _host_prep(query, key, value, Wq, bq, Wk, bk, Wv, bv, Wo, bo, pos_bias):
    """Build the 8 per-core input maps (host-side layout prep only)."""
    f = np.float32
    inv = f(1.0 / 32.0)  # 1/sqrt(1024), exact power of two
    wqt = np.ascontiguousarray(Wq.T.astype(f) * inv)
    wkt = np.ascontiguousarray(Wk.T.astype(f))
    wvt = np.ascontiguousarray(Wv.T.astype(f))
    wot = np.ascontiguousarray(Wo.T.astype(f))
    bq8 = np.ascontiguousarray((bq.astype(f) * inv).reshape(8, P).T)
    bk8 = np.ascontiguousarray(bk.astype(f).reshape(8, P).T)
    bvb = np.ascontiguousarray(np.broadcast_to(bv.astype(f), (P, D)))
    bob = np.ascontiguousarray(np.broadcast_to(bo.astype(f), (P, D)))
    pbc = np.ascontiguousarray(pos_bias.astype(f))
    shared = dict(wqt=wqt, wkt=wkt, wvt=wvt, wot=wot, bq8=bq8, bk8=bk8,
                  bvb=bvb, bob=bob, pb=pbc)
    in_maps = []
    for b in range(B):
        m = dict(shared)
        m["xq"] = np.ascontiguousarray(query[b].astype(f))
        m["xk"] = np.ascontiguousarray(key[b].astype(f))
        m["xv"] = np.ascontiguousarray(value[b].astype(f))
        in_maps.append(m)
    return in_maps


def kernel(query, key, value, Wq, bq, Wk, bk, Wv, bv, Wo, bo, pos_bias,
           _trace=False):
    from concourse.bass_utils import run_bass_kernel_spmd

    nc = _get_compiled()
    in_maps = _host_prep(np.asarray(query), np.asarray(key), np.asarray(value),
                         np.asarray(Wq), np.asarray(bq), np.asarray(Wk),
                         np.asarray(bk), np.asarray(Wv), np.asarray(bv),
                         np.asarray(Wo), np.asarray(bo), np.asarray(pos_bias))
    res = run_bass_kernel_spmd(nc, in_maps, core_ids=list(range(B)), trace=_trace)
    output = np.stack([res.results[b]["out"] for b in range(B)])
    attn_w = np.stack([res.results[b]["attn"] for b in range(B)])
    kernel.last_results = res
    return output, attn_w


# revision 12
# speedup vs baseline: 1.7224x; 1.7224x over previous
"""Trainium2 Bass kernel for nn_AttentionHead (B=8, S=2048, D=1024).

Sharding: data-parallel over batch — each of the 8 NeuronCores handles one
batch element; weights / biases / pos_bias are replicated.

Per-core dataflow (all matmuls run as float32r on the PE at 1 cyc/row):
  phase 1a: Q^T = (Wq/32) @ xq^T  + bq/32      -> spilled to DRAM scratch
  phase 1b: K^T = Wk @ xk^T + bk               -> SBUF resident [e, k]
  phase 1c: V   = xv @ Wv^T + bv               -> SBUF resident [k, e]
  phase 2 (per q-block of 256):
     S = Q^T.T @ K^T   (+pos_bias)  -> P = exp(S) (row-sum via accum_out)
     A = P * recip(rowsum)          -> DMA to attn_weights output
     A^T via PE transpose           -> O^T = V-tiles.T @ A^T -> DRAM scratch
  phase 3: out = O^T.T @ Wo^T + bo  -> output
x^T / A^T transposes use the PE transpose path (fp32, 2 cyc/row).
"""

import numpy as np

S = 2048
D = 1024
P = 128
B = 8

_COMPILED = None


def _build_program():
    import concourse.bass as bass
    import concourse.tile as tile
    from concourse import bacc, mybir
    from concourse.masks import make_identity
    from contextlib import ExitStack

    f32 = mybir.dt.float32
    f32r = mybir.dt.float32r
    AF = mybir.ActivationFunctionType

    nc = bacc.Bacc("TRN2", target_bir_lowering=False, debug=False)

    # ---- per-core I/O ----
    xq = nc.dram_tensor("xq", [S, D], f32, kind="ExternalInput").ap()
    xk = nc.dram_tensor("xk", [S, D], f32, kind="ExternalInput").ap()
    xv = nc.dram_tensor("xv", [S, D], f32, kind="ExternalInput").ap()
    wqt = nc.dram_tensor("wqt", [D, D], f32r, kind="ExternalInput").ap()  # Wq.T/32
    wkt = nc.dram_tensor("wkt", [D, D], f32r, kind="ExternalInput").ap()  # Wk.T
    wvt = nc.dram_tensor("wvt", [D, D], f32r, kind="ExternalInput").ap()  # Wv.T
    wot = nc.dram_tensor("wot", [D, D], f32r, kind="ExternalInput").ap()  # Wo.T
    bq8 = nc.dram_tensor("bq8", [P, 8], f32, kind="ExternalInput").ap()  # bq/32 [p, et]
    bk8 = nc.dram_tensor("bk8", [P, 8], f32, kind="ExternalInput").ap()
    bvb = nc.dram_tensor("bvb", [P, D], f32, kind="ExternalInput").ap()  # bv bcast
    bob = nc.dram_tensor("bob", [P, D], f32, kind="ExternalInput").ap()  # bo bcast
    pb = nc.dram_tensor("pb", [S, S], f32, kind="ExternalInput").ap()
    out = nc.dram_tensor("out", [S, D], f32, kind="ExternalOutput").ap()
    attn = nc.dram_tensor("attn", [S, S], f32, kind="ExternalOutput").ap()

    # DRAM scratch
    qt_d = nc.dram_tensor("qt_scratch", [D, S], f32r).ap()
    ot_d = nc.dram_tensor("ot_scratch", [D, S], f32r).ap()

    ET = D // P      # 8 e-tiles
    DT = D // P      # 8 d-tiles
    NSB = S // 512   # 4 s-blocks
    NQB = S // 256   # 8 q-blocks
    KT = S // P      # 16 k-tiles

    with tile.TileContext(nc) as tc, ExitStack() as top:
        const = top.enter_context(tc.tile_pool(name="const", bufs=1))
        mmp = top.enter_context(tc.tile_pool(name="mmp", bufs=4, space="PSUM"))
        tpp = top.enter_context(tc.tile_pool(name="tpp", bufs=2, space="PSUM"))

        ident = const.tile([P, P], f32, name="ident")
        make_identity(nc, ident[:])
        bq_sb = const.tile([P, 8], f32, name="bq_sb")
        nc.sync.dma_start(bq_sb[:], bq8[:])
        bk_sb = const.tile([P, 8], f32, name="bk_sb")
        nc.sync.dma_start(bk_sb[:], bk8[:])

        def load_xT(xp, xtp, x_dram, sb):
            """Load s-block sb of x and produce x^T tile [128, dt*512 + s]."""
            xts = []
            for ss in range(4):
                xt_ = xp.tile([P, D], f32, name=f"x_{sb}_{ss}", tag="x")
                nc.sync.dma_start(xt_[:], x_dram[(sb * 4 + ss) * P:(sb * 4 + ss + 1) * P, :])
                xts.append(xt_)
            xT = xtp.tile([P, DT * 512], f32r, name=f"xT_{sb}", tag="xT")
            for dt in range(DT):
                ps_t = tpp.tile([P, 512], f32, name="ps_t", tag="tp")
                for ss in range(4):
                    nc.tensor.transpose(
                        ps_t[:, ss * P:(ss + 1) * P],
                        xts[ss][:, dt * P:(dt + 1) * P],
                        ident[:],
                    )
                nc.vector.tensor_copy(xT[:, dt * 512:(dt + 1) * 512], ps_t[:])
            return xT

        # ---------------- phase 1a: Q^T -> DRAM ----------------
        with ExitStack() as ph:
            wp = ph.enter_context(tc.tile_pool(name="wp", bufs=1))
            xp = ph.enter_context(tc.tile_pool(name="xp", bufs=6))
            xtp = ph.enter_context(tc.tile_pool(name="xtp", bufs=2))
            qks = ph.enter_context(tc.tile_pool(name="qks", bufs=3))

            w_sb = wp.tile([P, DT * D], f32r, name="wq_sb", tag="w")
            for dt in range(DT):
                nc.sync.dma_start(w_sb[:, dt * D:(dt + 1) * D], wqt[dt * P:(dt + 1) * P, :])
            for sb in range(NSB):
                xT = load_xT(xp, xtp, xq, sb)
                for et in range(ET):
                    ps = mmp.tile([P, 512], f32, name="ps_q", tag="mm")
                    for dt in range(DT):
                        nc.tensor.matmul(
                            ps[:],
                            w_sb[:, dt * D + et * P: dt * D + (et + 1) * P],
                            xT[:, dt * 512:(dt + 1) * 512],
                            start=(dt == 0), stop=(dt == DT - 1),
                        )
                    stg = qks.tile([P, 512], f32r, name="qt_stage", tag="qs")
                    nc.scalar.activation(stg[:], ps[:], AF.Identity, bias=bq_sb[:, et:et + 1])
                    nc.sync.dma_start(qt_d[et * P:(et + 1) * P, sb * 512:(sb + 1) * 512], stg[:])

        # K^T resident from here to end of phase 2
        with ExitStack() as kv_scope:
            ktp = kv_scope.enter_context(tc.tile_pool(name="ktp", bufs=1))
            kt_res = ktp.tile([P, ET * S], f32r, name="kt_res")  # [e%128, et*2048+k]

            # ---------------- phase 1b: K^T -> resident ----------------
            with ExitStack() as ph:
                wp = ph.enter_context(tc.tile_pool(name="wp_k", bufs=1))
                xp = ph.enter_context(tc.tile_pool(name="xp_k", bufs=6))
                xtp = ph.enter_context(tc.tile_pool(name="xtp_k", bufs=2))

                w_sb = wp.tile([P, DT * D], f32r, name="wk_sb", tag="w")
                for dt in range(DT):
                    nc.sync.dma_start(w_sb[:, dt * D:(dt + 1) * D], wkt[dt * P:(dt + 1) * P, :])
                for sb in range(NSB):
                    xT = load_xT(xp, xtp, xk, sb)
                    for et in range(ET):
                        ps = mmp.tile([P, 512], f32, name="ps_k", tag="mm")
                        for dt in range(DT):
                            nc.tensor.matmul(
                                ps[:],
                                w_sb[:, dt * D + et * P: dt * D + (et + 1) * P],
                                xT[:, dt * 512:(dt + 1) * 512],
                                start=(dt == 0), stop=(dt == DT - 1),
                            )
                        nc.scalar.activation(
                            kt_res[:, et * S + sb * 512: et * S + (sb + 1) * 512],
                            ps[:], AF.Identity, bias=bk_sb[:, et:et + 1],
                        )

            # V resident from here to end of phase 2
            with ExitStack() as v_scope:
                vp = v_scope.enter_context(tc.tile_pool(name="vp", bufs=1))
                v_res = vp.tile([P, KT * D], f32r, name="v_res")  # [k%128, kt*1024+e]

                # ---------------- phase 1c: V -> resident ----------------
                with ExitStack() as ph:
                    wp = ph.enter_context(tc.tile_pool(name="wp_v", bufs=1))
                    xp = ph.enter_context(tc.tile_pool(name="xp_v", bufs=4))
                    xtp = ph.enter_context(tc.tile_pool(name="xtp_v", bufs=1))
                    bvp = ph.enter_context(tc.tile_pool(name="bvp", bufs=1))

                    w_sb = wp.tile([P, DT * D], f32r, name="wv_sb", tag="w")
                    for dt in range(DT):
                        nc.sync.dma_start(w_sb[:, dt * D:(dt + 1) * D],
                                          wvt[dt * P:(dt + 1) * P, :])
                    bv_sb = bvp.tile([P, D], f32, name="bv_sb")
                    nc.gpsimd.dma_start(bv_sb[:], bvb[:])
                    for sb in range(NSB):
                        xT = load_xT(xp, xtp, xv, sb)
                        for ss in range(4):
                            kt = sb * 4 + ss
                            for ec in range(2):
                                ps = mmp.tile([P, 512], f32, name="ps_v", tag="mm")
                                for dt in range(DT):
                                    nc.tensor.matmul(
                                        ps[:],
                                        xT[:, dt * 512 + ss * P: dt * 512 + (ss + 1) * P],
                                        w_sb[:, dt * D + ec * 512: dt * D + (ec + 1) * 512],
                                        start=(dt == 0), stop=(dt == DT - 1),
                                    )
                                nc.vector.tensor_add(
                                    v_res[:, kt * D + ec * 512: kt * D + (ec + 1) * 512],
                                    ps[:], bv_sb[:, ec * 512:(ec + 1) * 512],
                                )

                # ------------- phase 2: attention per q-block of 256 -------------
                with ExitStack() as ph:
                    qtb = ph.enter_context(tc.tile_pool(name="qtb", bufs=2))
                    pp = ph.enter_context(tc.tile_pool(name="pp", bufs=3))
                    pbp = ph.enter_context(tc.tile_pool(name="pbp", bufs=4))
                    atp = ph.enter_context(tc.tile_pool(name="atp", bufs=1))
                    otb = ph.enter_context(tc.tile_pool(name="otb", bufs=1))
                    smallp = ph.enter_context(tc.tile_pool(name="smallp", bufs=4))
                    opp = ph.enter_context(tc.tile_pool(name="opp", bufs=2, space="PSUM"))

                    for qb in range(NQB):
                        qtb_t = qtb.tile([P, ET * 256], f32r, name="qtb_t", tag="qtb")
                        for et in range(ET):
                            nc.sync.dma_start(
                                qtb_t[:, et * 256:(et + 1) * 256],
                                qt_d[et * P:(et + 1) * P, qb * 256:(qb + 1) * 256],
                            )
                        p_tiles = []
                        for qs in range(2):
                            q0 = qb * 256 + qs * P
                            p_t = pp.tile([P, S], f32, name=f"p_{qs}", tag="p")
                            for kc in range(4):
                                ps = mmp.tile([P, 512], f32, name="ps_s", tag="mm")
                                for et in range(ET):
                                    nc.tensor.matmul(
                                        ps[:],
                                        qtb_t[:, et * 256 + qs * P: et * 256 + (qs + 1) * P],
                                        kt_res[:, et * S + kc * 512: et * S + (kc + 1) * 512],
                                        start=(et == 0), stop=(et == ET - 1),
                                    )
                                pbt = pbp.tile([P, 512], f32, name="pbt", tag="pb")
                                nc.gpsimd.dma_start(pbt[:], pb[q0:q0 + P, kc * 512:(kc + 1) * 512])
                                nc.vector.tensor_add(p_t[:, kc * 512:(kc + 1) * 512], ps[:], pbt[:])
                            rs = smallp.tile([P, 1], f32, name="rs", tag="rs")
                            nc.scalar.activation(p_t[:], p_t[:], AF.Exp, accum_out=rs[:])
                            rc = smallp.tile([P, 1], f32, name="rc", tag="rc")
                            nc.vector.reciprocal(rc[:], rs[:])
                            nc.scalar.activation(p_t[:], p_t[:], AF.Identity, scale=rc[:])
                            nc.sync.dma_start(attn[q0:q0 + P, :], p_t[:])
                            p_tiles.append(p_t)

                        at_t = atp.tile([P, KT * 256], f32r, name="at_t", tag="at")
                        for kt in range(KT):
                            ps_t = tpp.tile([P, 512], f32, name="ps_at", tag="tp")
                            for qs in range(2):
                                nc.tensor.transpose(
                                    ps_t[:, qs * P:(qs + 1) * P],
                                    p_tiles[qs][:, kt * P:(kt + 1) * P],
                                    ident[:],
                                )
                            nc.vector.tensor_copy(at_t[:, kt * 256:(kt + 1) * 256], ps_t[:, 0:256])

                        otb_t = otb.tile([P, ET * 256], f32r, name="otb_t", tag="otb")
                        for et in range(ET):
                            ps = opp.tile([P, 256], f32, name="ps_o", tag="op")
                            for kt in range(KT):
                                nc.tensor.matmul(
                                    ps[:],
                                    v_res[:, kt * D + et * P: kt * D + (et + 1) * P],
                                    at_t[:, kt * 256:(kt + 1) * 256],
                                    start=(kt == 0), stop=(kt == KT - 1),
                                )
                            nc.vector.tensor_copy(otb_t[:, et * 256:(et + 1) * 256], ps[:])
                        for et in range(ET):
                            nc.gpsimd.dma_start(
                                ot_d[et * P:(et + 1) * P, qb * 256:(qb + 1) * 256],
                                otb_t[:, et * 256:(et + 1) * 256],
                            )

        # ---------------- phase 3: output projection ----------------
        with ExitStack() as ph:
            wop = ph.enter_context(tc.tile_pool(name="wop", bufs=1))
            bop = ph.enter_context(tc.tile_pool(name="bop", bufs=1))
            otl = ph.enter_context(tc.tile_pool(name="otl", bufs=3))
            outs = ph.enter_context(tc.tile_pool(name="outs", bufs=4))

            wo_sb = wop.tile([P, ET * D], f32r, name="wo_sb")
            for et in range(ET):
                nc.sync.dma_start(wo_sb[:, et * D:(et + 1) * D], wot[et * P:(et + 1) * P, :])
            bo_sb = bop.tile([P, D], f32, name="bo_sb")
            nc.gpsimd.dma_start(bo_sb[:], bob[:])
            for qp in range(NQB):  # q-pairs of 256
                ot_t = otl.tile([P, ET * 256], f32r, name="ot_t", tag="ot")
                for et in range(ET):
                    nc.gpsimd.dma_start(
                        ot_t[:, et * 256:(et + 1) * 256],
                        ot_d[et * P:(et + 1) * P, qp * 256:(qp + 1) * 256],
                    )
                for qs in range(2):
                    qt = qp * 2 + qs
                    for oc in range(2):
                        ps = mmp.tile([P, 512], f32, name="ps_f", tag="mm")
                        for et in range(ET):
                            nc.tensor.matmul(
                                ps[:],
                                ot_t[:, et * 256 + qs * P: et * 256 + (qs + 1) * P],
                                wo_sb[:, et * D + oc * 512: et * D + (oc + 1) * 512],
                                start=(et == 0), stop=(et == ET - 1),
                            )
                        os_t = outs.tile([P, 512], f32, name="os_t", tag="os")
                        nc.vector.tensor_add(os_t[:], ps[:], bo_sb[:, oc * 512:(oc + 1) * 512])
                        nc.sync.dma_start(out[qt * P:(qt + 1) * P, oc * 512:(oc + 1) * 512], os_t[:])

    nc.compile()
    return nc


def _get_compiled():
    global _COMPILED
    if _COMPILED is None:
        _COMPILED = _build_program()
    return _COMPILED


def _host_prep(query, key, value, Wq, bq, Wk, bk, Wv, bv, Wo, bo, pos_bias):
    """Build the 8 per-core input maps (host-side layout prep only)."""
    f = np.float32
    inv = f(1.0 / 32.0)  # 1/sqrt(1024), exact power of two
    wqt = np.ascontiguousarray(Wq.T.astype(f) * inv)
    wkt = np.ascontiguousarray(Wk.T.astype(f))
    wvt = np.ascontiguousarray(Wv.T.astype(f))
    wot = np.ascontiguousarray(Wo.T.astype(f))
    bq8 = np.ascontiguousarray((bq.astype(f) * inv).reshape(8, P).T)
    bk8 = np.ascontiguousarray(bk.astype(f).reshape(8, P).T)
    bvb = np.ascontiguousarray(np.broadcast_to(bv.astype(f), (P, D)))
    bob = np.ascontiguousarray(np.broadcast_to(bo.astype(f), (P, D)))
    pbc = np.ascontiguousarray(pos_bias.astype(f))
    shared = dict(wqt=wqt, wkt=wkt, wvt=wvt, wot=wot, bq8=bq8, bk8=bk8,
                  bvb=bvb, bob=bob, pb=pbc)
    in_maps = []
    for b in range(B):
        m = dict(shared)
        m["xq"] = np.ascontiguousarray(query[b].astype(f))
        m["xk"] = np.ascontiguousarray(key[b].astype(f))
        m["xv"] = np.ascontiguousarray(value[b].astype(f))
        in_maps.append(m)
    return in_maps


def kernel(query, key, value, Wq, bq, Wk, bk, Wv, bv, Wo, bo, pos_bias,
           _trace=False):
    from concourse.bass_utils import run_bass_kernel_spmd

    nc = _get_compiled()
    in_maps = _host_prep(np.asarray(query), np.asarray(key), np.asarray(value),
                         np.asarray(Wq), np.asarray(bq), np.asarray(Wk),
                         np.asarray(bk), np.asarray(Wv), np.asarray(bv),
                         np.asarray(Wo), np.asarray(bo), np.asarray(pos_bias))
    res = run_bass_kernel_spmd(nc, in_maps, core_ids=list(range(B)), trace=_trace)
    output = np.stack([res.results[b]["out"] for b in range(B)])
    attn_w = np.stack([res.results[b]["attn"] for b in range(B)])
    kernel.last_results = res
    return output, attn_w
